# revision 18
# baseline (speedup 1.0000x reference)
"""Spatial self-attention scores kernel for Trainium2 (8 NeuronCores).

Computes, per batch b:
    qk = W @ x_b          # [256, 4096] = [256,256] @ [256,4096]
    q, k = qk[:128], qk[128:]
    sim = (q.T @ k) * 128**-0.5
    out_b = softmax(sim, axis=-1)        # [4096, 4096]
Output: [8, 1, 4096, 4096] float32.

Sharding: data-parallel over batch, one batch image per NeuronCore.

The kernel is ScalarE-bound: softmax's exp runs only on the scalar
engine (1 elem/cycle/lane @ 1.2 GHz => ~109 us body for the 16.7M
outputs per core), so every other phase is arranged to hide under it:
  - x arrives via three HWDGE fp32 DMAs (W first, it is tiny) and is
    cast fp32->fp16 in chunks on the otherwise-idle GpSimd engine.
  - fp16 projection matmuls -> q,k in SBUF as [d=128, s=4096] fp16,
    interleaved with the first attention groups; extra PE warm-up
    matmuls keep the HAM clock ramp going while x lands.
  - per 128-query row-tile: fp16 matmuls (K=128, N=512) into 4-bank
    PSUM tiles; one ScalarE ACTIVATE per 2048 columns computes
    exp(SCALE*sim) straight to fp16 (no accum_out: the per-chunk row
    sums come from DVE tensor_reduce over the fp16 rows, which keeps
    the ~300ns ACTIVATION_READ_ACCUMULATOR off the bottleneck engine);
    DVE combines the partial sums, takes the reciprocal, scales the
    row.
  - the first row-tile runs 512/512/1024/2048-wide so the first
    ACTIVATE fires as soon as the first 512 columns of x land.
  - output leaves as fp16 (2 MB per two-row-tile transfer; first and
    last groups ship per normalized half-row) and is upcast to fp32 on
    the host. fp16 output halves the ~358 GB/s-per-core HBM write
    traffic that roofline-bound the fp32 version.
"""

import numpy as np
from contextlib import ExitStack

import concourse.bass as bass
import concourse.tile as tile
from concourse import bacc, mybir
from concourse.bass_utils import run_bass_kernel_spmd
from concourse.masks import make_identity

B = 8
C = 256
HW = 4096
D = 128
SCALE = D ** -0.5
N_CORES = 8

BANK = 512             # PSUM bank width (fp32) = one matmul free-dim
ACT_CHUNK = 2048       # one ScalarE activation spans 4 banks
N_ACT = HW // ACT_CHUNK          # 2
GRP = 2                # row-tiles per output DMA (2 -> 2 MB fp16 transfers)
N_GRP = HW // (128 * GRP)        # 16
OUT_BUFS = 4

F32 = mybir.dt.float32
F16 = mybir.dt.float16
MM_DT = mybir.dt.float16
PROJ_DT = mybir.dt.float16

# x input DMA chunks (fp32, HWDGE) and fp32->fp16 cast chunks (GpSimd);
# the first 512 columns land alone so the first row-tile can start.
X_DMA = ((0, 512), (512, 2048), (2048, 4096))
X_CAST = ((0, 512), (512, 1024), (1024, 2048),
          (2048, 2560), (2560, 3072), (3072, 3584), (3584, 4096))


def _emit(ctx: ExitStack, tc: tile.TileContext, out_ap, x_ap, w_ap):
    nc = tc.nc

    const = ctx.enter_context(tc.tile_pool(name="const", bufs=1))
    data = ctx.enter_context(tc.tile_pool(name="data", bufs=1))
    psum = ctx.enter_context(tc.tile_pool(name="psum", bufs=2, space="PSUM"))
    small = ctx.enter_context(tc.tile_pool(name="small", bufs=4))

    # ---- input DMAs on the SP HWDGE ring: first 512 x columns (they
    # start the whole pipeline), then W (tiny), then the rest of x.
    x_view = x_ap.rearrange("(t p) s -> p t s", p=128)
    x32_sb = data.tile([128, 2, HW], F32)
    nc.sync.dma_start(
        out=x32_sb[:, :, 0:512], in_=x_view[:, :, 0:512]
    )
    w_sb = const.tile([128, 2, C], F32)
    nc.sync.dma_start(out=w_sb, in_=w_ap.rearrange("(t p) c -> p t c", p=128))
    for lo, hi in X_DMA[1:]:
        nc.sync.dma_start(out=x32_sb[:, :, lo:hi], in_=x_view[:, :, lo:hi])

    # ---- PE warm-up: throwaway matmuls while x is loading. The PE
    # clock (HAM) only ramps after sustained activity; warming during
    # the input DMA makes the projection and the first attention
    # row-tiles run at full rate.
    warm_f32 = const.tile([128, BANK], F32)
    nc.vector.memset(warm_f32, 0.0)
    warm = const.tile([128, BANK], MM_DT)
    nc.vector.tensor_copy(out=warm, in_=warm_f32)
    wps = psum.tile([128, ACT_CHUNK], F32, tag="ps")
    for _ in range(4):
        nc.tensor.matmul(
            wps[:, 0:BANK], warm[:, 0:128], warm, start=True, stop=True
        )

    ident = const.tile([128, 128], F32)
    make_identity(nc, ident)

    # x fp32 -> fp16 casts on DVE (idle this early; GpSimd's software
    # copy is ~3x slower per element)
    x_sb = data.tile([128, 2, HW], PROJ_DT)
    for lo, hi in X_CAST:
        nc.vector.tensor_copy(out=x_sb[:, :, lo:hi], in_=x32_sb[:, :, lo:hi])

    # pull the exp table load off the first real activation
    tbl = small.tile([128, 1], F32, tag="tbl")
    nc.scalar.activation(
        out=tbl, in_=warm_f32[:, 0:1], func=mybir.ActivationFunctionType.Exp
    )

    # ---- transpose W on PE -> wt_sb[c_sub, c_tile, o] (contraction c on partitions)
    wt_sb = const.tile([128, 2, 2 * D], PROJ_DT)
    for t in range(2):          # output-channel tile (q half / k half)
        for ct in range(2):     # input-channel tile
            ps = psum.tile([128, ACT_CHUNK], F32, tag="ps")
            nc.tensor.transpose(
                ps[:, 0:128], w_sb[:, t, ct * 128:(ct + 1) * 128], ident
            )
            nc.vector.tensor_copy(
                out=wt_sb[:, ct, t * 128:(t + 1) * 128], in_=ps[:, 0:128]
            )
    # keep the PE clock ramping while the first x cast lands
    wps2 = psum.tile([128, ACT_CHUNK], F32, tag="ps")
    for _ in range(4):
        nc.tensor.matmul(
            wps2[:, 0:BANK], warm[:, 0:128], warm, start=True, stop=True
        )

    q_sb = data.tile([128, HW], MM_DT)
    k_sb = data.tile([128, HW], MM_DT)

    def proj_cols(t, dst, lo, hi):
        """Project output-channel half t (0=q, 1=k) for columns [lo, hi)
        (hi-lo <= 2048) in 512-wide banks."""
        ps = psum.tile([128, ACT_CHUNK], F32, tag="ps")
        for j in range((hi - lo) // BANK):
            sl = slice(lo + j * BANK, lo + (j + 1) * BANK)
            psl = slice(j * BANK, (j + 1) * BANK)
            for ct in range(2):
                nc.tensor.matmul(
                    ps[:, psl], wt_sb[:, ct, t * 128:(t + 1) * 128],
                    x_sb[:, ct, sl], start=(ct == 0), stop=(ct == 1),
                )
            nc.vector.tensor_copy(out=dst[:, sl], in_=ps[:, psl])

    outp = None
    out_view = out_ap.rearrange("(g t p) m -> g p t m", t=GRP, p=128)

    def sim_chunk(lhs, out_row, lo_col, n_col, accum, dve_sum=False):
        """n_col-wide slice of one attention row: matmuls + fused exp.

        The row sum comes either from the ACTIVATE's accumulator
        (costs an extra ~300ns ACTIVATION_READ_ACCUMULATOR on the
        bottleneck ScalarE; used early, while ScalarE still has gaps)
        or from an in-place DVE tensor_scalar pass (runs in 4x perf
        mode on the fp16 rows, ~755ns; used in steady state to keep
        ScalarE at pure-exp throughput)."""
        ps = psum.tile([128, ACT_CHUNK], F32, tag="ps")
        for jj in range(n_col // BANK):
            sl = slice(lo_col + jj * BANK, lo_col + (jj + 1) * BANK)
            nc.tensor.matmul(
                ps[:, jj * BANK:(jj + 1) * BANK], lhs, k_sb[:, sl],
                start=True, stop=True,
            )
        sl = slice(lo_col, lo_col + n_col)
        nc.scalar.activation(
            out=out_row[:, sl],
            in_=ps[:, 0:n_col],
            func=mybir.ActivationFunctionType.Exp,
            scale=SCALE,
            accum_out=None if dve_sum else accum,
        )
        if dve_sum:
            nc.vector.tensor_scalar(
                out=out_row[:, sl], in0=out_row[:, sl], scalar1=1.0,
                scalar2=0.0, op0=mybir.AluOpType.mult,
                op1=mybir.AluOpType.add, accum_out=accum,
            )

    def normalize_tile(out_grp, t, i, sums, split_dma):
        rsum = small.tile([128, 1], F32, tag="rsum")
        nc.vector.tensor_reduce(
            out=rsum, in_=sums, axis=mybir.AxisListType.X,
            op=mybir.AluOpType.add,
        )
        recip = small.tile([128, 1], F32, tag="recip")
        nc.vector.reciprocal(out=recip, in_=rsum)
        if split_dma:
            # normalize and ship each half-row as soon as it is scaled
            # (0.5 MB transfers): early groups -> earliest output bytes;
            # last group -> shortest tail.
            for a in range(N_ACT):
                sl = slice(a * ACT_CHUNK, (a + 1) * ACT_CHUNK)
                nc.vector.tensor_scalar_mul(
                    out=out_grp[:, t, sl], in0=out_grp[:, t, sl],
                    scalar1=recip,
                )
                nc.sync.dma_start(
                    out=out_ap[i * 128:(i + 1) * 128, sl],
                    in_=out_grp[:, t, sl],
                )
        else:
            nc.vector.tensor_scalar_mul(
                out=out_grp[:, t, :], in0=out_grp[:, t, :], scalar1=recip
            )

    def emit_group(g, split_dma=False):
        out_grp = outp.tile([128, GRP, HW], F16, tag="out")
        for t in range(GRP):
            i = g * GRP + t
            lhs = q_sb[:, i * 128:(i + 1) * 128]
            sums = small.tile([128, N_ACT], F32, tag="sums")
            for a in range(N_ACT):
                sim_chunk(lhs, out_grp[:, t], a * ACT_CHUNK, ACT_CHUNK,
                          sums[:, a:a + 1], dve_sum=True)
            normalize_tile(out_grp, t, i, sums, split_dma)
        if not split_dma:
            nc.sync.dma_start(out=out_view[g], in_=out_grp)

    def emit_early_groups():
        """Groups 0-1, reordered chunk-major: all four row-tiles' lower
        (cols 0:2048) chunks run first -- they only need the first half
        of x -- bridging ScalarE across the ~18 us it takes the upper
        half of x to arrive; the upper chunks and the normalizes follow.
        The very first row-tile runs 512/512/1024-wide so the first
        ACTIVATE fires as soon as the first 512 columns of x land."""
        og = [outp.tile([128, GRP, HW], F16, tag="out", name=f"og{j}")
              for j in range(2)]
        sums = [small.tile([128, 4 if i == 0 else 2], F32, tag="sums",
                           name=f"esums{i}")
                for i in range(4)]
        lhs = [q_sb[:, i * 128:(i + 1) * 128] for i in range(4)]
        # lower-half chunks (x cols 0:2048); k projection banks land
        # just ahead of the sim chunk that consumes them
        sim_chunk(lhs[0], og[0][:, 0], 0, 512, sums[0][:, 0:1])
        proj_cols(1, k_sb, 512, 1024)
        sim_chunk(lhs[0], og[0][:, 0], 512, 512, sums[0][:, 1:2])
        proj_cols(1, k_sb, 1024, 2048)
        sim_chunk(lhs[0], og[0][:, 0], 1024, 1024, sums[0][:, 2:3])
        sim_chunk(lhs[1], og[0][:, 1], 0, 2048, sums[1][:, 0:1])
        # spread the upper-half k projection between the remaining
        # lower chunks so the (HAM-throttled) PE never bursts
        proj_cols(1, k_sb, 2048, 3072)
        sim_chunk(lhs[2], og[1][:, 0], 0, 2048, sums[2][:, 0:1])
        proj_cols(1, k_sb, 3072, 4096)
        sim_chunk(lhs[3], og[1][:, 1], 0, 2048, sums[3][:, 0:1])
        # upper-half chunks + normalizes
        for i in range(4):
            last = 3 if i == 0 else 1
            sim_chunk(lhs[i], og[i // 2][:, i % 2], 2048, 2048,
                      sums[i][:, last:last + 1])
            normalize_tile(og[i // 2], i % 2, i, sums[i], True)

    # ---- projection, interleaved with the attention groups so the
    # in-order PE reaches the first ACTIVATE as early as possible.
    proj_cols(1, k_sb, 0, 512)      # k cols 0:512 (first x cast chunk)
    proj_cols(0, q_sb, 0, 512)      # q rows 0:512 -> groups 0-1

    outp = ctx.enter_context(tc.tile_pool(name="outp", bufs=OUT_BUFS))
    emit_early_groups()
    # remaining q projections trickle in one 512-wide bank at a time,
    # each just ahead of the first group that reads it
    proj_cols(0, q_sb, 512, 1024)    # rows  512:1024 (grps 2-3)
    emit_group(2)
    proj_cols(0, q_sb, 1024, 1536)   # rows 1024:1536 (grps 4-5)
    emit_group(3)
    proj_cols(0, q_sb, 1536, 2048)   # rows 1536:2048 (grps 6-7)
    for g in range(4, N_GRP // 2):
        emit_group(g)
        # q chunk 1 (row-tiles 16-31), one bank ahead of groups 8-11
        lo = 2048 + (g - 4) * BANK
        proj_cols(0, q_sb, lo, lo + BANK)
    for g in range(N_GRP // 2, N_GRP - 1):
        emit_group(g)
    emit_group(N_GRP - 1, split_dma=True)


_built = None


def _get_nc():
    global _built
    if _built is None:
        nc = bacc.Bacc("TRN2", target_bir_lowering=False, debug=False)
        x = nc.dram_tensor("x", [C, HW], F32, kind="ExternalInput").ap()
        w = nc.dram_tensor("w", [2 * D, C], F32, kind="ExternalInput").ap()
        out = nc.dram_tensor("out", [HW, HW], F16, kind="ExternalOutput").ap()
        with tile.TileContext(nc) as tc:
            with ExitStack() as ctx:
                _emit(ctx, tc, out, x, w)
        nc.compile()
        _built = nc
    return _built


def kernel(x: np.ndarray, W: np.ndarray) -> np.ndarray:
    nc = _get_nc()
    x = np.asarray(x, dtype=np.float32)
    W = np.ascontiguousarray(np.asarray(W, dtype=np.float32))
    in_maps = [
        {"x": np.ascontiguousarray(x[b].reshape(C, HW)), "w": W} for b in range(B)
    ]
    res = run_bass_kernel_spmd(nc, in_maps, core_ids=list(range(N_CORES)))
    out = np.stack(
        [res.results[b]["out"].astype(np.float32) for b in range(B)]
    )
    return out[:, None]


# revision 21
# speedup vs baseline: 1.3115x; 1.3115x over previous
"""Spatial self-attention scores kernel for Trainium2 (8 NeuronCores).

Computes, per batch b:
    qk = W @ x_b          # [256, 4096] = [256,256] @ [256,4096]
    q, k = qk[:128], qk[128:]
    sim = (q.T @ k) * 128**-0.5
    out_b = softmax(sim, axis=-1)        # [4096, 4096]
Output: [8, 1, 4096, 4096] float32.

Sharding: data-parallel over batch, one batch image per NeuronCore.

The kernel is ScalarE-bound: softmax's exp runs only on the scalar
engine (1 elem/cycle/lane @ 1.2 GHz => ~109 us body for the 16.7M
outputs per core), so every other phase is arranged to hide under it:
  - x arrives via three HWDGE fp32 DMAs (W first, it is tiny) and is
    cast fp32->fp16 in chunks on the otherwise-idle GpSimd engine.
  - fp16 projection matmuls -> q,k in SBUF as [d=128, s=4096] fp16,
    interleaved with the first attention groups; extra PE warm-up
    matmuls keep the HAM clock ramp going while x lands.
  - per 128-query row-tile: fp16 matmuls (K=128, N=512) into 4-bank
    PSUM tiles; one ScalarE ACTIVATE per 2048 columns computes
    exp(SCALE*sim) straight to fp16 (no accum_out: the per-chunk row
    sums come from DVE tensor_reduce over the fp16 rows, which keeps
    the ~300ns ACTIVATION_READ_ACCUMULATOR off the bottleneck engine);
    DVE combines the partial sums, takes the reciprocal, scales the
    row.
  - the first row-tile runs 512/512/1024/2048-wide so the first
    ACTIVATE fires as soon as the first 512 columns of x land.
  - output leaves as fp16 (2 MB per two-row-tile transfer; first and
    last groups ship per normalized half-row) and is upcast to fp32 on
    the host. fp16 output halves the ~358 GB/s-per-core HBM write
    traffic that roofline-bound the fp32 version.
"""

import numpy as np
from contextlib import ExitStack

import concourse.bass as bass
import concourse.tile as tile
from concourse import bacc, mybir
from concourse.bass_utils import run_bass_kernel_spmd
from concourse.masks import make_identity

B = 8
C = 256
HW = 4096
D = 128
SCALE = D ** -0.5
N_CORES = 8

BANK = 512             # PSUM bank width (fp32) = one matmul free-dim
ACT_CHUNK = 2048       # one ScalarE activation spans 4 banks
N_ACT = HW // ACT_CHUNK          # 2
GRP = 2                # row-tiles per output DMA (2 -> 2 MB fp16 transfers)
N_GRP = HW // (128 * GRP)        # 16
OUT_BUFS = 4

F32 = mybir.dt.float32
F16 = mybir.dt.float16
MM_DT = mybir.dt.float16
PROJ_DT = mybir.dt.float16

# x input DMA chunks (fp32, HWDGE) and fp32->fp16 cast chunks (GpSimd);
# the first 512 columns land alone so the first row-tile can start.
X_DMA = ((0, 512), (512, 2048), (2048, 4096))
X_CAST = ((0, 512), (512, 1024), (1024, 2048),
          (2048, 2560), (2560, 3072), (3072, 3584), (3584, 4096))


def _emit(ctx: ExitStack, tc: tile.TileContext, out_ap, x_ap, w_ap):
    nc = tc.nc

    const = ctx.enter_context(tc.tile_pool(name="const", bufs=1))
    data = ctx.enter_context(tc.tile_pool(name="data", bufs=1))
    psum = ctx.enter_context(tc.tile_pool(name="psum", bufs=2, space="PSUM"))
    small = ctx.enter_context(tc.tile_pool(name="small", bufs=4))

    # ---- input DMAs on the SP HWDGE ring: first 512 x columns (they
    # start the whole pipeline), then W (tiny), then the rest of x.
    x_view = x_ap.rearrange("(t p) s -> p t s", p=128)
    x32_sb = data.tile([128, 2, HW], F32)
    nc.sync.dma_start(
        out=x32_sb[:, :, 0:512], in_=x_view[:, :, 0:512]
    )
    w_sb = const.tile([128, 2, C], F32)
    nc.sync.dma_start(out=w_sb, in_=w_ap.rearrange("(t p) c -> p t c", p=128))
    for lo, hi in X_DMA[1:]:
        nc.sync.dma_start(out=x32_sb[:, :, lo:hi], in_=x_view[:, :, lo:hi])

    # ---- PE warm-up: throwaway matmuls while x is loading. The PE
    # clock (HAM) only ramps after sustained activity; warming during
    # the input DMA makes the projection and the first attention
    # row-tiles run at full rate.
    warm_f32 = const.tile([128, BANK], F32)
    nc.vector.memset(warm_f32, 0.0)
    warm = const.tile([128, BANK], MM_DT)
    nc.vector.tensor_copy(out=warm, in_=warm_f32)
    wps = psum.tile([128, ACT_CHUNK], F32, tag="ps")
    for _ in range(4):
        nc.tensor.matmul(
            wps[:, 0:BANK], warm[:, 0:128], warm, start=True, stop=True
        )

    ident = const.tile([128, 128], F32)
    make_identity(nc, ident)

    # x fp32 -> fp16 casts on DVE (idle this early; GpSimd's software
    # copy is ~3x slower per element)
    x_sb = data.tile([128, 2, HW], PROJ_DT)
    for lo, hi in X_CAST:
        nc.vector.tensor_copy(out=x_sb[:, :, lo:hi], in_=x32_sb[:, :, lo:hi])

    # pull the exp table load off the first real activation
    tbl = small.tile([128, 1], F32, tag="tbl")
    nc.scalar.activation(
        out=tbl, in_=warm_f32[:, 0:1], func=mybir.ActivationFunctionType.Exp
    )

    # ---- transpose W on PE -> wt_sb[c_sub, c_tile, o] (contraction c on partitions)
    wt_sb = const.tile([128, 2, 2 * D], PROJ_DT)
    for t in range(2):          # output-channel tile (q half / k half)
        for ct in range(2):     # input-channel tile
            ps = psum.tile([128, ACT_CHUNK], F32, tag="ps")
            nc.tensor.transpose(
                ps[:, 0:128], w_sb[:, t, ct * 128:(ct + 1) * 128], ident
            )
            nc.vector.tensor_copy(
                out=wt_sb[:, ct, t * 128:(t + 1) * 128], in_=ps[:, 0:128]
            )
    # keep the PE clock ramping while the first x cast lands
    wps2 = psum.tile([128, ACT_CHUNK], F32, tag="ps")
    for _ in range(4):
        nc.tensor.matmul(
            wps2[:, 0:BANK], warm[:, 0:128], warm, start=True, stop=True
        )

    q_sb = data.tile([128, HW], MM_DT)
    k_sb = data.tile([128, HW], MM_DT)

    def proj_cols(t, dst, lo, hi):
        """Project output-channel half t (0=q, 1=k) for columns [lo, hi)
        (hi-lo <= 2048) in 512-wide banks."""
        ps = psum.tile([128, ACT_CHUNK], F32, tag="ps")
        for j in range((hi - lo) // BANK):
            sl = slice(lo + j * BANK, lo + (j + 1) * BANK)
            psl = slice(j * BANK, (j + 1) * BANK)
            for ct in range(2):
                nc.tensor.matmul(
                    ps[:, psl], wt_sb[:, ct, t * 128:(t + 1) * 128],
                    x_sb[:, ct, sl], start=(ct == 0), stop=(ct == 1),
                )
            nc.vector.tensor_copy(out=dst[:, sl], in_=ps[:, psl])

    outp = None
    out_view = out_ap.rearrange("(g t p) m -> g p t m", t=GRP, p=128)

    def sim_chunk(lhs, out_row, lo_col, n_col, accum):
        """n_col-wide slice of one attention row: matmuls + fused exp.

        Row sums come from the ACTIVATE's accumulator. The extra
        ~300ns ACTIVATION_READ_ACCUMULATOR per chunk stays on ScalarE
        deliberately: every DVE-side alternative measures worse
        (tensor_reduce only reaches 2x perf mode = 2.2us/chunk;
        tensor_scalar with accum_out lowers to TENSOR_SCALAR_CACHE_
        REDUCE at 2.7us/chunk)."""
        ps = psum.tile([128, ACT_CHUNK], F32, tag="ps")
        for jj in range(n_col // BANK):
            sl = slice(lo_col + jj * BANK, lo_col + (jj + 1) * BANK)
            nc.tensor.matmul(
                ps[:, jj * BANK:(jj + 1) * BANK], lhs, k_sb[:, sl],
                start=True, stop=True,
            )
        sl = slice(lo_col, lo_col + n_col)
        nc.scalar.activation(
            out=out_row[:, sl],
            in_=ps[:, 0:n_col],
            func=mybir.ActivationFunctionType.Exp,
            scale=SCALE,
            accum_out=accum,
        )

    def normalize_tile(out_grp, t, i, sums, split_dma):
        rsum = small.tile([128, 1], F32, tag="rsum")
        nc.vector.tensor_reduce(
            out=rsum, in_=sums, axis=mybir.AxisListType.X,
            op=mybir.AluOpType.add,
        )
        recip = small.tile([128, 1], F32, tag="recip")
        nc.vector.reciprocal(out=recip, in_=rsum)
        if split_dma:
            # normalize and ship each half-row as soon as it is scaled
            # (0.5 MB transfers): early groups -> earliest output bytes;
            # last group -> shortest tail.
            for a in range(N_ACT):
                sl = slice(a * ACT_CHUNK, (a + 1) * ACT_CHUNK)
                nc.vector.tensor_scalar_mul(
                    out=out_grp[:, t, sl], in0=out_grp[:, t, sl],
                    scalar1=recip,
                )
                nc.sync.dma_start(
                    out=out_ap[i * 128:(i + 1) * 128, sl],
                    in_=out_grp[:, t, sl],
                )
        else:
            nc.vector.tensor_scalar_mul(
                out=out_grp[:, t, :], in0=out_grp[:, t, :], scalar1=recip
            )

    def emit_group(g, split_dma=False):
        out_grp = outp.tile([128, GRP, HW], F16, tag="out")
        for t in range(GRP):
            i = g * GRP + t
            lhs = q_sb[:, i * 128:(i + 1) * 128]
            sums = small.tile([128, N_ACT], F32, tag="sums")
            for a in range(N_ACT):
                sim_chunk(lhs, out_grp[:, t], a * ACT_CHUNK, ACT_CHUNK,
                          sums[:, a:a + 1])
            normalize_tile(out_grp, t, i, sums, split_dma)
        if not split_dma:
            nc.sync.dma_start(out=out_view[g], in_=out_grp)

    def emit_early_groups():
        """Groups 0-1, reordered chunk-major: all four row-tiles' lower
        (cols 0:2048) chunks run first -- they only need the first half
        of x -- bridging ScalarE across the ~18 us it takes the upper
        half of x to arrive; the upper chunks and the normalizes follow.
        The very first row-tile runs 512/512/1024-wide so the first
        ACTIVATE fires as soon as the first 512 columns of x land."""
        og = [outp.tile([128, GRP, HW], F16, tag="out", name=f"og{j}")
              for j in range(2)]
        sums = [small.tile([128, 4 if i == 0 else 2], F32, tag="sums",
                           name=f"esums{i}")
                for i in range(4)]
        lhs = [q_sb[:, i * 128:(i + 1) * 128] for i in range(4)]
        # lower-half chunks (x cols 0:2048); k projection banks land
        # just ahead of the sim chunk that consumes them
        sim_chunk(lhs[0], og[0][:, 0], 0, 512, sums[0][:, 0:1])
        proj_cols(1, k_sb, 512, 1024)
        sim_chunk(lhs[0], og[0][:, 0], 512, 512, sums[0][:, 1:2])
        proj_cols(1, k_sb, 1024, 2048)
        sim_chunk(lhs[0], og[0][:, 0], 1024, 1024, sums[0][:, 2:3])
        sim_chunk(lhs[1], og[0][:, 1], 0, 2048, sums[1][:, 0:1])
        # spread the upper-half k projection between the remaining
        # lower chunks so the (HAM-throttled) PE never bursts
        proj_cols(1, k_sb, 2048, 3072)
        sim_chunk(lhs[2], og[1][:, 0], 0, 2048, sums[2][:, 0:1])
        proj_cols(1, k_sb, 3072, 4096)
        sim_chunk(lhs[3], og[1][:, 1], 0, 2048, sums[3][:, 0:1])
        # upper-half chunks + normalizes
        for i in range(4):
            last = 3 if i == 0 else 1
            sim_chunk(lhs[i], og[i // 2][:, i % 2], 2048, 2048,
                      sums[i][:, last:last + 1])
            normalize_tile(og[i // 2], i % 2, i, sums[i], True)

    # ---- projection, interleaved with the attention groups so the
    # in-order PE reaches the first ACTIVATE as early as possible.
    proj_cols(1, k_sb, 0, 512)      # k cols 0:512 (first x cast chunk)
    proj_cols(0, q_sb, 0, 512)      # q rows 0:512 -> groups 0-1

    outp = ctx.enter_context(tc.tile_pool(name="outp", bufs=OUT_BUFS))
    emit_early_groups()
    # remaining q projections trickle in one 512-wide bank at a time,
    # each just ahead of the first group that reads it
    proj_cols(0, q_sb, 512, 1024)    # rows  512:1024 (grps 2-3)
    emit_group(2)
    proj_cols(0, q_sb, 1024, 1536)   # rows 1024:1536 (grps 4-5)
    emit_group(3)
    proj_cols(0, q_sb, 1536, 2048)   # rows 1536:2048 (grps 6-7)
    for g in range(4, N_GRP // 2):
        emit_group(g)
        # q chunk 1 (row-tiles 16-31), one bank ahead of groups 8-11
        lo = 2048 + (g - 4) * BANK
        proj_cols(0, q_sb, lo, lo + BANK)
    for g in range(N_GRP // 2, N_GRP - 1):
        emit_group(g)
    emit_group(N_GRP - 1, split_dma=True)


_built = None


def _get_nc():
    global _built
    if _built is None:
        nc = bacc.Bacc("TRN2", target_bir_lowering=False, debug=False)
        x = nc.dram_tensor("x", [C, HW], F32, kind="ExternalInput").ap()
        w = nc.dram_tensor("w", [2 * D, C], F32, kind="ExternalInput").ap()
        out = nc.dram_tensor("out", [HW, HW], F16, kind="ExternalOutput").ap()
        with tile.TileContext(nc) as tc:
            with ExitStack() as ctx:
                _emit(ctx, tc, out, x, w)
        nc.compile()
        _built = nc
    return _built


def kernel(x: np.ndarray, W: np.ndarray) -> np.ndarray:
    nc = _get_nc()
    x = np.asarray(x, dtype=np.float32)
    W = np.ascontiguousarray(np.asarray(W, dtype=np.float32))
    in_maps = [
        {"x": np.ascontiguousarray(x[b].reshape(C, HW)), "w": W} for b in range(B)
    ]
    res = run_bass_kernel_spmd(nc, in_maps, core_ids=list(range(N_CORES)))
    out = np.stack(
        [res.results[b]["out"].astype(np.float32) for b in range(B)]
    )
    return out[:, None]


# revision 30
# speedup vs baseline: 1.5671x; 1.1949x over previous
"""Spatial self-attention scores kernel for Trainium2 (8 NeuronCores).

Computes, per batch b:
    qk = W @ x_b          # [256, 4096] = [256,256] @ [256,4096]
    q, k = qk[:128], qk[128:]
    sim = (q.T @ k) * 128**-0.5
    out_b = softmax(sim, axis=-1)        # [4096, 4096]
Output: [8, 1, 4096, 4096] float32.

Sharding: data-parallel over batch, one batch image per NeuronCore.

The kernel is ScalarE-bound: softmax's exp runs only on the scalar
engine (1 elem/cycle/lane @ 1.2 GHz => ~109 us body for the 16.7M
outputs per core), so every other phase is arranged to hide under it:
  - x arrives via three HWDGE fp32 DMAs (W first, it is tiny) and is
    cast fp32->fp16 in chunks on the otherwise-idle GpSimd engine.
  - fp16 projection matmuls -> q,k in SBUF as [d=128, s=4096] fp16,
    interleaved with the first attention groups; extra PE warm-up
    matmuls keep the HAM clock ramp going while x lands.
  - per 128-query row-tile: fp16 matmuls (K=128, N=512) into 4-bank
    PSUM tiles; one ScalarE ACTIVATE per 2048 columns computes
    exp(SCALE*sim) straight to fp16 (no accum_out: the per-chunk row
    sums come from DVE tensor_reduce over the fp16 rows, which keeps
    the ~300ns ACTIVATION_READ_ACCUMULATOR off the bottleneck engine);
    DVE combines the partial sums, takes the reciprocal, scales the
    row.
  - the first row-tile runs 512/512/1024/2048-wide so the first
    ACTIVATE fires as soon as the first 512 columns of x land.
  - output leaves as fp16 (2 MB per two-row-tile transfer; first and
    last groups ship per normalized half-row) and is upcast to fp32 on
    the host. fp16 output halves the ~358 GB/s-per-core HBM write
    traffic that roofline-bound the fp32 version.
"""

import numpy as np
from contextlib import ExitStack

import concourse.bass as bass
import concourse.tile as tile
from concourse import bacc, mybir
from concourse.bass_utils import run_bass_kernel_spmd
from concourse.masks import make_identity

B = 8
C = 256
HW = 4096
D = 128
SCALE = D ** -0.5
N_CORES = 8

BANK = 512             # PSUM bank width (fp32) = one matmul free-dim
ACT_CHUNK = 2048       # one ScalarE activation spans 4 banks
N_ACT = HW // ACT_CHUNK          # 2
GRP = 2                # row-tiles per output DMA (2 -> 2 MB fp16 transfers)
N_GRP = HW // (128 * GRP)        # 16
OUT_BUFS = 4

F32 = mybir.dt.float32
F16 = mybir.dt.float16
MM_DT = mybir.dt.float16
PROJ_DT = mybir.dt.float16

# x input DMA chunks (fp32, HWDGE) and fp32->fp16 cast chunks (GpSimd);
# the first 512 columns land alone so the first row-tile can start.
X_DMA = ((0, 512), (512, 2048), (2048, 4096))
X_CAST = ((0, 512), (512, 1024), (1024, 2048),
          (2048, 2560), (2560, 3072), (3072, 3584), (3584, 4096))


def _emit(ctx: ExitStack, tc: tile.TileContext, out_ap, x_ap, w_ap):
    nc = tc.nc

    const = ctx.enter_context(tc.tile_pool(name="const", bufs=1))
    data = ctx.enter_context(tc.tile_pool(name="data", bufs=1))
    psum = ctx.enter_context(tc.tile_pool(name="psum", bufs=2, space="PSUM"))
    small = ctx.enter_context(tc.tile_pool(name="small", bufs=4))

    # ---- input DMAs on the SP HWDGE ring: first 512 x columns (they
    # start the whole pipeline), then W (tiny), then the rest of x.
    x_view = x_ap.rearrange("(t p) s -> p t s", p=128)
    x32_sb = data.tile([128, 2, HW], F32)
    nc.sync.dma_start(
        out=x32_sb[:, :, 0:512], in_=x_view[:, :, 0:512]
    )
    w_sb = const.tile([128, 2, C], F32)
    nc.sync.dma_start(out=w_sb, in_=w_ap.rearrange("(t p) c -> p t c", p=128))
    for lo, hi in X_DMA[1:]:
        nc.sync.dma_start(out=x32_sb[:, :, lo:hi], in_=x_view[:, :, lo:hi])

    # ---- PE warm-up: throwaway matmuls while x is loading. The PE
    # clock (HAM) only ramps after sustained activity; warming during
    # the input DMA makes the projection and the first attention
    # row-tiles run at full rate.
    warm_f32 = const.tile([128, BANK], F32)
    nc.vector.memset(warm_f32, 0.0)
    warm = const.tile([128, BANK], MM_DT)
    nc.vector.tensor_copy(out=warm, in_=warm_f32)
    wps = psum.tile([128, ACT_CHUNK], F32, tag="ps")
    for _ in range(4):
        nc.tensor.matmul(
            wps[:, 0:BANK], warm[:, 0:128], warm, start=True, stop=True
        )

    ident = const.tile([128, 128], F32)
    make_identity(nc, ident)

    # x fp32 -> fp16 casts on DVE (idle this early; GpSimd's software
    # copy is ~3x slower per element)
    x_sb = data.tile([128, 2, HW], PROJ_DT)
    for lo, hi in X_CAST:
        nc.vector.tensor_copy(out=x_sb[:, :, lo:hi], in_=x32_sb[:, :, lo:hi])

    # pull the exp table load off the first real activation; the dummy
    # accum_out read resets the ACT accumulator register so the warm-up
    # exp(0)=1 does not leak into the first row's sum
    tbl = small.tile([128, 2], F32, tag="tbl")
    nc.scalar.activation(
        out=tbl[:, 0:1], in_=warm_f32[:, 0:1],
        func=mybir.ActivationFunctionType.Exp, accum_out=tbl[:, 1:2],
    )

    # ---- transpose W on PE -> wt_sb[c_sub, c_tile, o] (contraction c on partitions)
    wt_sb = const.tile([128, 2, 2 * D], PROJ_DT)
    for t in range(2):          # output-channel tile (q half / k half)
        for ct in range(2):     # input-channel tile
            ps = psum.tile([128, ACT_CHUNK], F32, tag="ps")
            nc.tensor.transpose(
                ps[:, 0:128], w_sb[:, t, ct * 128:(ct + 1) * 128], ident
            )
            nc.vector.tensor_copy(
                out=wt_sb[:, ct, t * 128:(t + 1) * 128], in_=ps[:, 0:128]
            )
    # keep the PE clock ramping while the first x cast lands
    wps2 = psum.tile([128, ACT_CHUNK], F32, tag="ps")
    for _ in range(4):
        nc.tensor.matmul(
            wps2[:, 0:BANK], warm[:, 0:128], warm, start=True, stop=True
        )

    q_sb = data.tile([128, HW], MM_DT)
    k_sb = data.tile([128, HW], MM_DT)

    def proj_cols(t, dst, lo, hi):
        """Project output-channel half t (0=q, 1=k) for columns [lo, hi)
        (hi-lo <= 2048) in 512-wide banks."""
        ps = psum.tile([128, ACT_CHUNK], F32, tag="ps")
        for j in range((hi - lo) // BANK):
            sl = slice(lo + j * BANK, lo + (j + 1) * BANK)
            psl = slice(j * BANK, (j + 1) * BANK)
            for ct in range(2):
                nc.tensor.matmul(
                    ps[:, psl], wt_sb[:, ct, t * 128:(t + 1) * 128],
                    x_sb[:, ct, sl], start=(ct == 0), stop=(ct == 1),
                )
            nc.vector.tensor_copy(out=dst[:, sl], in_=ps[:, psl])

    outp = None
    out_view = out_ap.rearrange("(g t p) m -> g p t m", t=GRP, p=128)

    def sim_chunk(lhs, out_row, lo_col, n_col, accum):
        """n_col-wide slice of one attention row: matmuls + fused exp.

        Row sums come from the ACTIVATE's per-instruction accumulator
        (accumulation does NOT persist across ACTIVATEs -- measured),
        so every chunk pays its ~300ns ACTIVATION_READ_ACCUMULATOR on
        ScalarE. That is still the cheapest option: DVE-side
        alternatives all measure worse (tensor_reduce: 2x mode =
        2.2us/chunk; tensor_scalar with accum_out lowers to
        TENSOR_SCALAR_CACHE_REDUCE at 2.7us/chunk)."""
        ps = psum.tile([128, ACT_CHUNK], F32, tag="ps")
        for jj in range(n_col // BANK):
            sl = slice(lo_col + jj * BANK, lo_col + (jj + 1) * BANK)
            nc.tensor.matmul(
                ps[:, jj * BANK:(jj + 1) * BANK], lhs, k_sb[:, sl],
                start=True, stop=True,
            )
        sl = slice(lo_col, lo_col + n_col)
        nc.scalar.activation(
            out=out_row[:, sl],
            in_=ps[:, 0:n_col],
            func=mybir.ActivationFunctionType.Exp,
            scale=SCALE,
            accum_out=accum,
        )

    def normalize_tile(out_grp, t, i, rsum, split_dma):
        recip = small.tile([128, 1], F32, tag="recip")
        nc.vector.reciprocal(out=recip, in_=rsum)
        if split_dma:
            # normalize and ship each half-row as soon as it is scaled
            # (0.5 MB transfers): early groups -> earliest output bytes;
            # last group -> shortest tail.
            for a in range(N_ACT):
                sl = slice(a * ACT_CHUNK, (a + 1) * ACT_CHUNK)
                nc.vector.tensor_scalar_mul(
                    out=out_grp[:, t, sl], in0=out_grp[:, t, sl],
                    scalar1=recip,
                )
                nc.sync.dma_start(
                    out=out_ap[i * 128:(i + 1) * 128, sl],
                    in_=out_grp[:, t, sl],
                )
        else:
            nc.vector.tensor_scalar_mul(
                out=out_grp[:, t, :], in0=out_grp[:, t, :], scalar1=recip
            )

    def emit_group(g, split_dma=False, fine_tail=False):
        out_grp = outp.tile([128, GRP, HW], F16, tag="out")
        for t in range(GRP):
            i = g * GRP + t
            lhs = q_sb[:, i * 128:(i + 1) * 128]
            fine = fine_tail and t == GRP - 1
            n_sum = 3 if fine else N_ACT
            sums = small.tile([128, n_sum], F32, tag="sums")
            sim_chunk(lhs, out_grp[:, t], 0, ACT_CHUNK, sums[:, 0:1])
            if fine:
                # split the very last chunk so the final exp->normalize->
                # DMA tail is as short as possible
                sim_chunk(lhs, out_grp[:, t], ACT_CHUNK, 1024,
                          sums[:, 1:2])
                sim_chunk(lhs, out_grp[:, t], ACT_CHUNK + 1024, 1024,
                          sums[:, 2:3])
            else:
                sim_chunk(lhs, out_grp[:, t], ACT_CHUNK, ACT_CHUNK,
                          sums[:, 1:2])
            rsum = small.tile([128, 1], F32, tag="rsum")
            nc.vector.tensor_reduce(
                out=rsum, in_=sums, axis=mybir.AxisListType.X,
                op=mybir.AluOpType.add,
            )
            normalize_tile(out_grp, t, i, rsum, split_dma)
        if not split_dma:
            nc.sync.dma_start(out=out_view[g], in_=out_grp)

    def emit_early_groups():
        """Groups 0-1, reordered chunk-major: all four row-tiles' lower
        (cols 0:2048) chunks run first -- they only need the first half
        of x -- bridging ScalarE across the ~18 us it takes the upper
        half of x to arrive; the upper chunks and the normalizes follow.
        The very first row-tile runs 512/512/1024-wide so the first
        ACTIVATE fires as soon as the first 512 columns of x land."""
        og = [outp.tile([128, GRP, HW], F16, tag="out", name=f"og{j}")
              for j in range(2)]
        sums = [small.tile([128, 4 if i == 0 else 2], F32, tag="sums",
                           name=f"esums{i}")
                for i in range(4)]
        lhs = [q_sb[:, i * 128:(i + 1) * 128] for i in range(4)]
        # Early tiles interleave chunk-major (all lower chunks first,
        # bridging ScalarE across the upper-x DMA), so the cross-
        # ACTIVATE accumulator trick cannot be used here: every chunk
        # reads its own accumulator. The extra READs land in ScalarE's
        # ramp-up gaps, where they are free.
        # lower-half chunks (x cols 0:2048); k projection banks land
        # just ahead of the sim chunk that consumes them.
        sim_chunk(lhs[0], og[0][:, 0], 0, 512, sums[0][:, 0:1])
        proj_cols(1, k_sb, 512, 1024)
        sim_chunk(lhs[0], og[0][:, 0], 512, 512, sums[0][:, 1:2])
        proj_cols(1, k_sb, 1024, 2048)
        sim_chunk(lhs[0], og[0][:, 0], 1024, 1024, sums[0][:, 2:3])
        sim_chunk(lhs[1], og[0][:, 1], 0, 2048, sums[1][:, 0:1])
        # spread the upper-half k projection between the remaining
        # lower chunks so the (HAM-throttled) PE never bursts
        proj_cols(1, k_sb, 2048, 3072)
        sim_chunk(lhs[2], og[1][:, 0], 0, 2048, sums[2][:, 0:1])
        proj_cols(1, k_sb, 3072, 4096)
        sim_chunk(lhs[3], og[1][:, 1], 0, 2048, sums[3][:, 0:1])
        # upper-half chunks + normalizes
        for i in range(4):
            last = 3 if i == 0 else 1
            sim_chunk(lhs[i], og[i // 2][:, i % 2], 2048, 2048,
                      sums[i][:, last:last + 1])
            rsum = small.tile([128, 1], F32, tag="rsum")
            nc.vector.tensor_reduce(
                out=rsum, in_=sums[i], axis=mybir.AxisListType.X,
                op=mybir.AluOpType.add,
            )
            normalize_tile(og[i // 2], i % 2, i, rsum, True)

    # ---- projection, interleaved with the attention groups so the
    # in-order PE reaches the first ACTIVATE as early as possible.
    proj_cols(1, k_sb, 0, 512)      # k cols 0:512 (first x cast chunk)
    proj_cols(0, q_sb, 0, 512)      # q rows 0:512 -> groups 0-1

    outp = ctx.enter_context(tc.tile_pool(name="outp", bufs=OUT_BUFS))
    emit_early_groups()
    # remaining q projections trickle in one 512-wide bank at a time,
    # each just ahead of the first group that reads it
    proj_cols(0, q_sb, 512, 1024)    # rows  512:1024 (grps 2-3)
    emit_group(2)
    proj_cols(0, q_sb, 1024, 1536)   # rows 1024:1536 (grps 4-5)
    emit_group(3)
    proj_cols(0, q_sb, 1536, 2048)   # rows 1536:2048 (grps 6-7)
    for g in range(4, N_GRP // 2):
        emit_group(g)
        # q chunk 1 (row-tiles 16-31), one bank ahead of groups 8-11
        lo = 2048 + (g - 4) * BANK
        proj_cols(0, q_sb, lo, lo + BANK)
    for g in range(N_GRP // 2, N_GRP - 1):
        emit_group(g)
    emit_group(N_GRP - 1, split_dma=True, fine_tail=True)


_built = None


def _get_nc():
    global _built
    if _built is None:
        nc = bacc.Bacc("TRN2", target_bir_lowering=False, debug=False)
        x = nc.dram_tensor("x", [C, HW], F32, kind="ExternalInput").ap()
        w = nc.dram_tensor("w", [2 * D, C], F32, kind="ExternalInput").ap()
        out = nc.dram_tensor("out", [HW, HW], F16, kind="ExternalOutput").ap()
        with tile.TileContext(nc) as tc:
            with ExitStack() as ctx:
                _emit(ctx, tc, out, x, w)
        nc.compile()
        _built = nc
    return _built


def kernel(x: np.ndarray, W: np.ndarray) -> np.ndarray:
    nc = _get_nc()
    x = np.asarray(x, dtype=np.float32)
    W = np.ascontiguousarray(np.asarray(W, dtype=np.float32))
    in_maps = [
        {"x": np.ascontiguousarray(x[b].reshape(C, HW)), "w": W} for b in range(B)
    ]
    res = run_bass_kernel_spmd(nc, in_maps, core_ids=list(range(N_CORES)))
    out = np.stack(
        [res.results[b]["out"].astype(np.float32) for b in range(B)]
    )
    return out[:, None]
